# revision 11
# baseline (speedup 1.0000x reference)
"""Trainium2 Bass kernel for nn_DecodeSBP (keypoint heatmap decode).

Contract: kernel(x=[1,133,512,512] f32) -> [133,3] f32
  joints[k] = (4*xx, 4*yy, conf) if conf > 0.8 else (-4, -4, -1)
  where flat = argmax(sigmoid(x[0,k])), conf = sigmoid(max), yy = flat//512,
  xx = flat%512. sigmoid is monotonic so the argmax runs on raw logits.

Sharding: keypoint dim across 8 cores (17/core, core 7 zero-padded).
Per-core program: stream 17 MB through SBUF with one DVE reduce_max pass
(per-partition max per keypoint) -> TensorE transpose -> global max ->
mask*reversed-iota argmax idiom for winning partition -> gather of the
17 winning 8 KB rows (register-offset DMAs) -> same idiom for the index within the row -> decode.
"""

import sys
from contextlib import ExitStack

for _p in ("/opt/trn_rl_repo", "/opt/pypackages"):
    if _p not in sys.path:
        sys.path.append(_p)

import numpy as np

import concourse.bacc as bacc
import concourse.bass as bass
import concourse.tile as tile
from concourse import mybir
from concourse.bass_utils import run_bass_kernel_spmd
from concourse.masks import make_identity

K = 17          # keypoints per core
NK = 133        # total keypoints
ROW = 262144    # 512*512
P = 128         # SBUF partitions
F = ROW // P    # 2048 free elems per partition
W = 512
N_CORES = 8

f32 = mybir.dt.float32
i16 = mybir.dt.int16
Alu = mybir.AluOpType
Act = mybir.ActivationFunctionType

_NC_CACHE = None


def _build(kp_per_tile=3, bufs=6):
    nc = bacc.Bacc("TRN2", target_bir_lowering=False, debug=False)
    x_dram = nc.dram_tensor("x", [K, ROW], f32, kind="ExternalInput")
    out_dram = nc.dram_tensor("out", [K, 3], f32, kind="ExternalOutput")

    x_pkf = x_dram.ap().rearrange("k (p f) -> p k f", f=F)      # [128, K, 2048]

    with tile.TileContext(nc) as tc, ExitStack() as ctx:
        const_pool = ctx.enter_context(tc.tile_pool(name="const", bufs=1))
        in_pool = ctx.enter_context(tc.tile_pool(name="in", bufs=bufs))
        small_pool = ctx.enter_context(tc.tile_pool(name="small", bufs=1))
        psum_pool = ctx.enter_context(
            tc.tile_pool(name="psum", bufs=1, space="PSUM"))

        ident = const_pool.tile([P, P], f32)
        make_identity(nc, ident[:])
        # riota_p[k, j] = 128 - j
        riota_p = const_pool.tile([K, P], f32)
        nc.gpsimd.iota(riota_p[:], pattern=[[-1, P]], base=P,
                       channel_multiplier=0,
                       allow_small_or_imprecise_dtypes=True)
        # riota_j[k, j] = 2048 - j
        riota_j = const_pool.tile([K, F], f32)
        nc.gpsimd.iota(riota_j[:], pattern=[[-1, F]], base=F,
                       channel_multiplier=0,
                       allow_small_or_imprecise_dtypes=True)
        # phase 1: stream all data, per-partition max per keypoint
        pmax = small_pool.tile([P, K], f32)
        k0 = 0
        while k0 < K:
            g = min(kp_per_tile, K - k0)
            t = in_pool.tile([P, g * F], f32, tag="xin")
            nc.gpsimd.dma_start(
                t[:].rearrange("p (g f) -> p g f", f=F),
                x_pkf[:, k0:k0 + g, :])
            nc.vector.reduce_max(
                pmax[:, k0:k0 + g],
                t[:].rearrange("p (g f) -> p g f", f=F),
                axis=mybir.AxisListType.X)
            k0 += g

        # phase 2: transpose pmax -> pT [K, 128]
        psumT = psum_pool.tile([K, P], f32)
        nc.tensor.matmul(psumT[:], pmax[:], ident[:], is_transpose=True)
        pT = small_pool.tile([K, P], f32)
        nc.vector.tensor_copy(pT[:], psumT[:])

        # phase 3: global max + winning partition per keypoint
        gmax = small_pool.tile([K, 1], f32)
        nc.vector.reduce_max(gmax[:], pT[:], axis=mybir.AxisListType.X)
        cand_p = small_pool.tile([K, P], f32)
        nc.vector.scalar_tensor_tensor(
            cand_p[:], in0=pT[:], scalar=gmax[:], in1=riota_p[:],
            op0=Alu.is_ge, op1=Alu.mult)
        rp = small_pool.tile([K, 1], f32)     # rp = 128 - p*
        nc.vector.reduce_max(rp[:], cand_p[:], axis=mybir.AxisListType.X)

        # phase 4: gather the winning row of each keypoint from DRAM via
        # 17 register-offset DMAs (dma_gather's custom DGE op is unavailable
        # in this runtime; plain dynamic DMA with an explicit sem works).
        # element offset of row k = k*262144 + p* * 2048 = (128*(k+1) - rp)*2048
        psumR = psum_pool.tile([1, K], f32, tag="psumR")
        nc.tensor.matmul(psumR[:], rp[:], ident[0:K, 0:K], is_transpose=True)
        offs_f = small_pool.tile([1, K], f32)
        # kiota_row[0, k] = 128*(k+1)
        kiota_row = const_pool.tile([1, K], f32)
        nc.gpsimd.iota(kiota_row[:], pattern=[[P, K]], base=P,
                       channel_multiplier=0,
                       allow_small_or_imprecise_dtypes=True)
        nc.vector.tensor_sub(offs_f[:], kiota_row[:], psumR[:])
        nc.vector.tensor_scalar(offs_f[:], offs_f[:], float(F), None, Alu.mult)
        offs_i = small_pool.tile([1, K], mybir.dt.int32)
        nc.vector.tensor_copy(offs_i[:], offs_f[:])

        x_flat = x_dram.ap().rearrange("k f -> (k f)")
        grow = small_pool.tile([K, F], f32)
        dsem = nc.alloc_semaphore("gather_dma")
        with tc.tile_critical():
            for k in range(K):
                reg = nc.sync.alloc_register()
                nc.sync.load(reg, offs_i[0:1, k:k + 1])
                off = nc.sync.snap(reg, donate=True)
                nc.sync.dma_start(
                    grow[k:k + 1, :],
                    x_flat[bass.ds(off, F)]).then_inc(dsem, 16)
            nc.sync.wait_ge(dsem, K * 16)

        # phase 5: index within the winning row
        cand_j = small_pool.tile([K, F], f32)
        nc.vector.scalar_tensor_tensor(
            cand_j[:], in0=grow[:], scalar=gmax[:], in1=riota_j[:],
            op0=Alu.is_ge, op1=Alu.mult)
        rj = small_pool.tile([K, 1], f32)     # rj = 2048 - j*
        nc.vector.reduce_max(rj[:], cand_j[:], axis=mybir.AxisListType.X)

        # phase 6: decode coordinates + confidence
        p_f = small_pool.tile([K, 1], f32)    # p* = 128 - rp
        nc.vector.tensor_scalar(p_f[:], rp[:], -1.0, float(P), Alu.mult, Alu.add)
        j_f = small_pool.tile([K, 1], f32)    # j* = 2048 - rj
        nc.vector.tensor_scalar(j_f[:], rj[:], -1.0, float(F), Alu.mult, Alu.add)
        flat = small_pool.tile([K, 1], f32)   # flat = p* * 2048 + j*
        nc.vector.scalar_tensor_tensor(
            flat[:], in0=p_f[:], scalar=float(F), in1=j_f[:],
            op0=Alu.mult, op1=Alu.add)
        i32 = mybir.dt.int32
        flat_i = small_pool.tile([K, 1], i32)
        nc.vector.tensor_copy(flat_i[:], flat[:])
        xx_i = small_pool.tile([K, 1], i32)   # flat & 511
        nc.vector.tensor_scalar(xx_i[:], flat_i[:], W - 1, None,
                                Alu.bitwise_and)
        yy_i = small_pool.tile([K, 1], i32)   # flat >> 9
        nc.vector.tensor_scalar(yy_i[:], flat_i[:], 9, None,
                                Alu.logical_shift_right)
        xx = small_pool.tile([K, 1], f32)
        nc.vector.tensor_copy(xx[:], xx_i[:])
        yy = small_pool.tile([K, 1], f32)
        nc.vector.tensor_copy(yy[:], yy_i[:])

        conf = small_pool.tile([K, 1], f32)
        nc.scalar.activation(conf[:], gmax[:], Act.Sigmoid)
        valid = small_pool.tile([K, 1], f32)
        nc.vector.tensor_scalar(valid[:], conf[:], 0.8, None, Alu.is_gt)

        # out = valid * (val + ofs) - ofs ; ofs = 4 for x,y and 1 for conf
        out_sb = small_pool.tile([K, 3], f32)
        tmp = small_pool.tile([K, 1], f32, tag="tmp")
        nc.vector.tensor_scalar(tmp[:], xx[:], 4.0, 4.0, Alu.mult, Alu.add)
        nc.vector.tensor_mul(tmp[:], tmp[:], valid[:])
        nc.vector.tensor_scalar(out_sb[:, 0:1], tmp[:], 4.0, None, Alu.subtract)
        tmp2 = small_pool.tile([K, 1], f32, tag="tmp2")
        nc.vector.tensor_scalar(tmp2[:], yy[:], 4.0, 4.0, Alu.mult, Alu.add)
        nc.vector.tensor_mul(tmp2[:], tmp2[:], valid[:])
        nc.vector.tensor_scalar(out_sb[:, 1:2], tmp2[:], 4.0, None, Alu.subtract)
        tmp3 = small_pool.tile([K, 1], f32, tag="tmp3")
        nc.vector.tensor_scalar(tmp3[:], conf[:], 1.0, None, Alu.add)
        nc.vector.tensor_mul(tmp3[:], tmp3[:], valid[:])
        nc.vector.tensor_scalar(out_sb[:, 2:3], tmp3[:], 1.0, None, Alu.subtract)

        nc.sync.dma_start(out_dram.ap(), out_sb[:])

    nc.compile()
    return nc


def _get_nc():
    global _NC_CACHE
    if _NC_CACHE is None:
        _NC_CACHE = _build()
    return _NC_CACHE


def _shard(x: np.ndarray) -> list[dict[str, np.ndarray]]:
    xf = np.ascontiguousarray(np.asarray(x, dtype=np.float32).reshape(NK, ROW))
    shards = []
    for c in range(N_CORES):
        lo = c * K
        s = xf[lo:min(lo + K, NK)]
        if s.shape[0] < K:
            s = np.concatenate(
                [s, np.zeros((K - s.shape[0], ROW), np.float32)], axis=0)
        shards.append({"x": np.ascontiguousarray(s)})
    return shards


def _run(x, trace=False, **kw):
    nc = _get_nc()
    res = run_bass_kernel_spmd(nc, _shard(x), core_ids=list(range(N_CORES)),
                               trace=trace, **kw)
    out = np.concatenate([r["out"] for r in res.results], axis=0)[:NK]
    return out.astype(np.float32), res


def kernel(x: np.ndarray) -> np.ndarray:
    out, _ = _run(x, trace=False)
    return out


# revision 14
# speedup vs baseline: 1.0801x; 1.0801x over previous
"""Trainium2 Bass kernel for nn_DecodeSBP (keypoint heatmap decode).

Contract: kernel(x=[1,133,512,512] f32) -> [133,3] f32
  joints[k] = (4*xx, 4*yy, conf) if conf > 0.8 else (-4, -4, -1)
  where flat = argmax(sigmoid(x[0,k])), conf = sigmoid(max), yy = flat//512,
  xx = flat%512. sigmoid is monotonic so the argmax runs on raw logits.

Sharding: keypoint dim across 8 cores (17/core, core 7 zero-padded).
Per-core program: stream 17 MB through SBUF with one DVE reduce_max pass
(per-partition max per keypoint) -> TensorE transpose -> global max ->
mask*reversed-iota argmax idiom for winning partition -> gather of the
17 winning 8 KB rows (register-offset DMAs) -> same idiom for the index within the row -> decode.
"""

import sys
from contextlib import ExitStack

for _p in ("/opt/trn_rl_repo", "/opt/pypackages"):
    if _p not in sys.path:
        sys.path.append(_p)

import numpy as np

import concourse.bacc as bacc
import concourse.bass as bass
import concourse.tile as tile
from concourse import mybir
from concourse.bass_utils import run_bass_kernel_spmd
from concourse.masks import make_identity

K = 17          # keypoints per core
NK = 133        # total keypoints
ROW = 262144    # 512*512
P = 128         # SBUF partitions
F = ROW // P    # 2048 free elems per partition
W = 512
N_CORES = 8

f32 = mybir.dt.float32
i16 = mybir.dt.int16
Alu = mybir.AluOpType
Act = mybir.ActivationFunctionType

_NC_CACHE = None


def _build(kp_per_tile=3, bufs=6):
    nc = bacc.Bacc("TRN2", target_bir_lowering=False, debug=False)
    x_dram = nc.dram_tensor("x", [K, ROW], f32, kind="ExternalInput")
    out_dram = nc.dram_tensor("out", [K, 3], f32, kind="ExternalOutput")

    x_pkf = x_dram.ap().rearrange("k (p f) -> p k f", f=F)      # [128, K, 2048]

    with tile.TileContext(nc) as tc, ExitStack() as ctx:
        const_pool = ctx.enter_context(tc.tile_pool(name="const", bufs=1))
        in_pool = ctx.enter_context(tc.tile_pool(name="in", bufs=bufs))
        small_pool = ctx.enter_context(tc.tile_pool(name="small", bufs=1))
        psum_pool = ctx.enter_context(
            tc.tile_pool(name="psum", bufs=1, space="PSUM"))

        ident = const_pool.tile([P, P], f32)
        make_identity(nc, ident[:])
        # riota_p[k, j] = 128 - j
        riota_p = const_pool.tile([K, P], f32)
        nc.gpsimd.iota(riota_p[:], pattern=[[-1, P]], base=P,
                       channel_multiplier=0,
                       allow_small_or_imprecise_dtypes=True)
        # riota_j[k, j] = 2048 - j
        riota_j = const_pool.tile([K, F], f32)
        nc.gpsimd.iota(riota_j[:], pattern=[[-1, F]], base=F,
                       channel_multiplier=0,
                       allow_small_or_imprecise_dtypes=True)
        # phase 1: stream all data, per-partition max per keypoint
        pmax = small_pool.tile([P, K], f32)
        k0 = 0
        while k0 < K:
            g = min(kp_per_tile, K - k0)
            t = in_pool.tile([P, g * F], f32, tag="xin")
            nc.sync.dma_start(
                t[:].rearrange("p (g f) -> p g f", f=F),
                x_pkf[:, k0:k0 + g, :])
            nc.vector.reduce_max(
                pmax[:, k0:k0 + g],
                t[:].rearrange("p (g f) -> p g f", f=F),
                axis=mybir.AxisListType.X)
            k0 += g

        # phase 2: transpose pmax -> pT [K, 128]
        psumT = psum_pool.tile([K, P], f32)
        nc.tensor.matmul(psumT[:], pmax[:], ident[:], is_transpose=True)
        pT = small_pool.tile([K, P], f32)
        nc.vector.tensor_copy(pT[:], psumT[:])

        # phase 3: global max + winning partition per keypoint
        gmax = small_pool.tile([K, 1], f32)
        nc.vector.reduce_max(gmax[:], pT[:], axis=mybir.AxisListType.X)
        cand_p = small_pool.tile([K, P], f32)
        nc.vector.scalar_tensor_tensor(
            cand_p[:], in0=pT[:], scalar=gmax[:], in1=riota_p[:],
            op0=Alu.is_ge, op1=Alu.mult)
        rp = small_pool.tile([K, 1], f32)     # rp = 128 - p*
        nc.vector.reduce_max(rp[:], cand_p[:], axis=mybir.AxisListType.X)

        # phase 4: gather the winning row of each keypoint from DRAM via
        # 17 register-offset DMAs (dma_gather's custom DGE op is unavailable
        # in this runtime; plain dynamic DMA with an explicit sem works).
        # element offset of row k = k*262144 + p* * 2048 = (128*(k+1) - rp)*2048
        psumR = psum_pool.tile([1, K], f32, tag="psumR")
        nc.tensor.matmul(psumR[:], rp[:], ident[0:K, 0:K], is_transpose=True)
        offs_f = small_pool.tile([1, K], f32)
        # kiota_row[0, k] = 128*(k+1)
        kiota_row = const_pool.tile([1, K], f32)
        nc.gpsimd.iota(kiota_row[:], pattern=[[P, K]], base=P,
                       channel_multiplier=0,
                       allow_small_or_imprecise_dtypes=True)
        nc.vector.tensor_sub(offs_f[:], kiota_row[:], psumR[:])
        nc.vector.tensor_scalar(offs_f[:], offs_f[:], float(F), None, Alu.mult)
        offs_i = small_pool.tile([1, K], mybir.dt.int32)
        nc.vector.tensor_copy(offs_i[:], offs_f[:])

        x_flat = x_dram.ap().rearrange("k f -> (k f)")
        grow = small_pool.tile([K, F], f32)
        dsem = nc.alloc_semaphore("gather_dma")
        # 17 dynamic DMAs cost ~1.1us each on one sequencer; spread them
        # over four engines so the gather takes ~5us instead of ~18us.
        engines = [nc.sync, nc.scalar, nc.gpsimd]
        with tc.tile_critical():
            for k in range(K):
                eng = engines[k % len(engines)]
                reg = eng.alloc_register()
                eng.load(reg, offs_i[0:1, k:k + 1])
                off = eng.snap(reg, donate=True)
                eng.dma_start(
                    grow[k:k + 1, :],
                    x_flat[bass.ds(off, F)]).then_inc(dsem, 16)
            nc.sync.wait_ge(dsem, K * 16)

        # phase 5: index within the winning row
        cand_j = small_pool.tile([K, F], f32)
        nc.vector.scalar_tensor_tensor(
            cand_j[:], in0=grow[:], scalar=gmax[:], in1=riota_j[:],
            op0=Alu.is_ge, op1=Alu.mult)
        rj = small_pool.tile([K, 1], f32)     # rj = 2048 - j*
        nc.vector.reduce_max(rj[:], cand_j[:], axis=mybir.AxisListType.X)

        # phase 6: decode coordinates + confidence
        p_f = small_pool.tile([K, 1], f32)    # p* = 128 - rp
        nc.vector.tensor_scalar(p_f[:], rp[:], -1.0, float(P), Alu.mult, Alu.add)
        j_f = small_pool.tile([K, 1], f32)    # j* = 2048 - rj
        nc.vector.tensor_scalar(j_f[:], rj[:], -1.0, float(F), Alu.mult, Alu.add)
        flat = small_pool.tile([K, 1], f32)   # flat = p* * 2048 + j*
        nc.vector.scalar_tensor_tensor(
            flat[:], in0=p_f[:], scalar=float(F), in1=j_f[:],
            op0=Alu.mult, op1=Alu.add)
        i32 = mybir.dt.int32
        flat_i = small_pool.tile([K, 1], i32)
        nc.vector.tensor_copy(flat_i[:], flat[:])
        xx_i = small_pool.tile([K, 1], i32)   # flat & 511
        nc.vector.tensor_scalar(xx_i[:], flat_i[:], W - 1, None,
                                Alu.bitwise_and)
        yy_i = small_pool.tile([K, 1], i32)   # flat >> 9
        nc.vector.tensor_scalar(yy_i[:], flat_i[:], 9, None,
                                Alu.logical_shift_right)
        xx = small_pool.tile([K, 1], f32)
        nc.vector.tensor_copy(xx[:], xx_i[:])
        yy = small_pool.tile([K, 1], f32)
        nc.vector.tensor_copy(yy[:], yy_i[:])

        conf = small_pool.tile([K, 1], f32)
        nc.scalar.activation(conf[:], gmax[:], Act.Sigmoid)
        valid = small_pool.tile([K, 1], f32)
        nc.vector.tensor_scalar(valid[:], conf[:], 0.8, None, Alu.is_gt)

        # out = valid * (val + ofs) - ofs ; ofs = 4 for x,y and 1 for conf
        out_sb = small_pool.tile([K, 3], f32)
        tmp = small_pool.tile([K, 1], f32, tag="tmp")
        nc.vector.tensor_scalar(tmp[:], xx[:], 4.0, 4.0, Alu.mult, Alu.add)
        nc.vector.tensor_mul(tmp[:], tmp[:], valid[:])
        nc.vector.tensor_scalar(out_sb[:, 0:1], tmp[:], 4.0, None, Alu.subtract)
        tmp2 = small_pool.tile([K, 1], f32, tag="tmp2")
        nc.vector.tensor_scalar(tmp2[:], yy[:], 4.0, 4.0, Alu.mult, Alu.add)
        nc.vector.tensor_mul(tmp2[:], tmp2[:], valid[:])
        nc.vector.tensor_scalar(out_sb[:, 1:2], tmp2[:], 4.0, None, Alu.subtract)
        tmp3 = small_pool.tile([K, 1], f32, tag="tmp3")
        nc.vector.tensor_scalar(tmp3[:], conf[:], 1.0, None, Alu.add)
        nc.vector.tensor_mul(tmp3[:], tmp3[:], valid[:])
        nc.vector.tensor_scalar(out_sb[:, 2:3], tmp3[:], 1.0, None, Alu.subtract)

        nc.sync.dma_start(out_dram.ap(), out_sb[:])

    nc.compile()
    return nc


def _get_nc():
    global _NC_CACHE
    if _NC_CACHE is None:
        _NC_CACHE = _build()
    return _NC_CACHE


def _shard(x: np.ndarray) -> list[dict[str, np.ndarray]]:
    xf = np.ascontiguousarray(np.asarray(x, dtype=np.float32).reshape(NK, ROW))
    shards = []
    for c in range(N_CORES):
        lo = c * K
        s = xf[lo:min(lo + K, NK)]
        if s.shape[0] < K:
            s = np.concatenate(
                [s, np.zeros((K - s.shape[0], ROW), np.float32)], axis=0)
        shards.append({"x": np.ascontiguousarray(s)})
    return shards


def _run(x, trace=False, **kw):
    nc = _get_nc()
    res = run_bass_kernel_spmd(nc, _shard(x), core_ids=list(range(N_CORES)),
                               trace=trace, **kw)
    out = np.concatenate([r["out"] for r in res.results], axis=0)[:NK]
    return out.astype(np.float32), res


def kernel(x: np.ndarray) -> np.ndarray:
    out, _ = _run(x, trace=False)
    return out
